# revision 1
# baseline (speedup 1.0000x reference)
"""Bass/Tile TRN2 kernel for nn_DecoderGroupedQueryHeadAttentionAlibi.

Sharding (8 cores): core = (b, g) with b = core//2 in [0,4) (batch),
g = core%2 (head-half). Each core computes 8 of 16 query heads (those with
h%4 in {2g, 2g+1}) for its batch, plus the corresponding row-slice of the
output projection; the host sums the two half partials and adds bproj.

Per-core device program (layout A, scoresT = [s_partitions, t_free]):
  - emission order keeps the ACT (exp) engine — the bottleneck — streaming:
    head-0 attention starts right after k + q-pair-0; v and q pairs 1-3 are
    interleaved into the head stream; the Toeplitz exp table is built
    on-device via stride-0 broadcast DMA (32KB of HBM instead of 4MB)
  - per (head, s-tile): scoresT psum [128,1024] tiles -> ACT exp (alibi bias
    folded into the per-partition activation bias) -> DVE multiplier applied
    per region (past/diag/future) -> attn@v psum accumulation where row 64
    (a ones column in v, padded to 128 weight columns so FWL stays enabled)
    is the softmax denominator
  - per-head epilogue (hidden under the next head): copy out, reciprocal of
    the denominator, broadcast, normalize outT; final output projection.

The alibi bias of this module is min(a_h*(s-t), 0) (tril overwrites the causal
mask in the torch reference, so future tokens are attended with bias 0), hence
P = exp(score/8) * min(exp(a*(s-t)), 1), which factors into a per-partition
ACT bias exp(a*(s_in-127)) and a distance-only (Toeplitz) row multiplier
exp(-a*delta). Score columns with t - s > margin/a are dropped (banded): the
dropped softmax mass is < e^-margin of the kept band mass.
"""

import math
import numpy as np

# ---- problem constants (hardcoded; kernel.py must be self-contained) ----
B, T, C = 4, 2048, 1024
N_HEAD, N_KV_HEAD, HEAD_DIM = 16, 4, 64
NH = 8            # heads per core
ST = T // 128     # 16 s-tiles
NCH = T // 512    # 4 t-chunks
KCT = C // 128    # 8 contraction tiles of 128
WREP_W = 2048     # Toeplitz table width: index = t - 128*j is always < 2048
MARGIN = 7.0      # exp(-7) ~ 9e-4: dropped mass is < 1e-3 of the band mass

_START = 2.0 ** (-2.0 ** (-(math.log2(N_HEAD) - 3.0)))  # 0.7071...


def _head_of_slot(i: int, g: int) -> int:
    return 4 * (i // 2) + 2 * g + (i % 2)


def _a_of_head(h: int) -> float:
    return (_START ** (h + 1)) / math.sqrt(HEAD_DIM)


# Loop bounds must be identical on every core (SPMD): use the widest cutoff
# over g for each head slot (g=1 heads have smaller slopes -> wider bands).
_CUTOFF = [MARGIN / min(_a_of_head(_head_of_slot(i, 0)),
                        _a_of_head(_head_of_slot(i, 1)))
           for i in range(NH)]
_R128 = [min(T, int(math.ceil(c / 128.0)) * 128) for c in _CUTOFF]
# 128-granular computed width per (head slot, s-tile): scores/exp/multiplies
_W128 = [[min(T, 128 * (j + 1) + _R128[i]) for j in range(ST)]
         for i in range(NH)]
# 512-granular width for the attn@v accumulation (E tail is zeroed)
_W512 = [[((w + 511) // 512) * 512 for w in row] for row in _W128]
_NEFF = [[w // 512 for w in row] for row in _W512]
_J_FIRST = [[min(j for j in range(ST) if _NEFF[i][j] > tcn)
             for tcn in range(NCH)] for i in range(NH)]

_NC_CACHE = {}


def _split_multiwait(nc, mybir, max_waits=1):
    """walrus in this env encodes at most one sync-wait per instruction;
    split extras onto same-engine NoOps emitted just before."""
    for f in nc.m.functions:
        for bb in f.blocks:
            new = []
            for ins in bb.instructions:
                si = ins.sync_info
                conds = list(si.on_wait) if si is not None else []
                if len(conds) > max_waits:
                    for cond in conds[:-max_waits]:
                        n = mybir.InstNoOp(
                            name=nc.get_next_instruction_name(), ins=[], outs=[])
                        n.engine = ins.engine
                        n.sync_info = mybir.SyncInfo(on_wait=[cond], on_update=[])
                        new.append(n)
                    si.on_wait = conds[-max_waits:]
                new.append(ins)
            bb.instructions = new


def _build_nc():
    if "nc" in _NC_CACHE:
        return _NC_CACHE["nc"]
    import concourse.bass as bass
    import concourse.tile as tile
    from concourse import mybir

    f32 = mybir.dt.float32
    bf16 = mybir.dt.bfloat16
    AF = mybir.ActivationFunctionType
    MUL = mybir.AluOpType.mult
    MIN = mybir.AluOpType.min

    nc = bass.Bass()

    xT_d = nc.dram_tensor("xT", [C, T], bf16, kind="ExternalInput")
    wq_d = nc.dram_tensor("wqT", [C, NH * 64], bf16, kind="ExternalInput")
    wk_d = nc.dram_tensor("wkT", [C, 128], bf16, kind="ExternalInput")
    wv_d = nc.dram_tensor("wvT", [C, 128], bf16, kind="ExternalInput")
    wp_d = nc.dram_tensor("wpT", [NH * 64, C], bf16, kind="ExternalInput")
    wrow_d = nc.dram_tensor("wrow", [NH, WREP_W], bf16, kind="ExternalInput")
    u_d = nc.dram_tensor("usb", [128, NH], f32, kind="ExternalInput")
    bias_d = nc.dram_tensor("biassb", [128, NH], f32, kind="ExternalInput")
    out_d = nc.dram_tensor("out", [T, C], f32, kind="ExternalOutput")

    xT_r = xT_d.rearrange("(k p) t -> p k t", p=128)
    wq_r = wq_d.rearrange("(k p) e -> p k e", p=128)

    def bcast(src_row, parts):
        # [1, W] DRAM row -> [parts, W] stride-0 partition broadcast source
        return bass.AP(tensor=src_row.tensor, offset=src_row.offset,
                       ap=[[0, parts]] + list(src_row.ap)[1:])

    with tile.TileContext(nc) as tc:
        with (
            tc.tile_pool(name="const", bufs=1) as const,
            tc.tile_pool(name="work", bufs=3) as work,
            tc.tile_pool(name="ebuf", bufs=3) as ebufp,
            tc.tile_pool(name="stp", bufs=2) as stp,
            tc.tile_pool(name="rrp", bufs=2) as rrp,
            tc.tile_pool(name="dpk", bufs=4) as dpk,
            tc.tile_pool(name="outp", bufs=2) as outp,
            tc.tile_pool(name="dramd", bufs=1, space="DRAM") as dramd,
        ):
            # ---- persistent tiles ----
            kRep = const.tile([128, 2, T], bf16)     # kv on both halves
            v_sb = const.tile([128, ST, 130], bf16)  # [s, j, (v_kv0|1|v_kv1|1)]
            qRep = const.tile([128, NH, T], bf16)    # head i on both halves
            outT = const.tile([128, 4, T], bf16)     # [(2 heads d), pair, t]
            wrep = const.tile([128, NH, WREP_W], bf16)
            wp = const.tile([128, 4, C], bf16)
            usb = const.tile([128, NH], f32)
            biassb = const.tile([128, NH], f32)
            xT = const.tile([128, KCT, T], bf16)
            wq = const.tile([128, KCT, NH * 64], bf16)
            wk = const.tile([128, KCT, 128], bf16)
            wv = const.tile([128, KCT, 128], bf16)
            warm = const.tile([128, 1], f32)
            wsink = const.tile([128, 1], f32)
            drow_d = dramd.tile([NH, T], bf16)
            rrow_d = dramd.tile([NH, T], bf16)

            # ---- ACT exp-table preload (runs during the DMA ramp) ----
            nc.vector.memset(warm, 0.0)
            nc.scalar.activation(wsink, warm, AF.Exp, scale=1.0)

            # ---- input DMAs (per-queue issue order is the priority) ----
            for kc in range(KCT):
                eng = nc.sync if kc < 4 else nc.scalar
                eng.dma_start(out=xT[:, kc, :], in_=xT_r[:, kc, :])
            nc.gpsimd.dma_start(out=wk, in_=wk_d.rearrange("(k p) e -> p k e", p=128))
            nc.gpsimd.dma_start(out=usb, in_=u_d[:])
            nc.gpsimd.dma_start(out=biassb, in_=bias_d[:])
            wrow_r = [wrow_d[i:i + 1, :] for i in range(NH)]

            def wrep_bc(i):
                nc.gpsimd.dma_start(out=wrep[:, i, :], in_=bcast(wrow_r[i], 128))

            wrep_bc(0)
            wrep_bc(1)
            for kc in range(KCT):
                nc.gpsimd.dma_start(out=wq[:, kc, :], in_=wq_r[:, kc, :])
            nc.gpsimd.dma_start(out=wv, in_=wv_d.rearrange("(k p) e -> p k e", p=128))
            for i in range(2, NH):
                wrep_bc(i)
            nc.gpsimd.dma_start(out=wp, in_=wp_d.rearrange("(k p) e -> p k e", p=128))

            with (
                tc.tile_pool(name="psS", bufs=2, space="PSUM") as psS,
                tc.tile_pool(name="psA", bufs=1, space="PSUM") as psAp,
            ):
                # ---- projection emitters (share the psS psum pool) ----
                def k_proj():
                    for sh in range(2):
                        ps = psS.tile([128, 1024], f32, tag="S", name=f"kp{sh}")
                        for sub in range(2):
                            sc = 2 * sh + sub
                            for kc in range(KCT):
                                nc.tensor.matmul(
                                    ps[:, 512 * sub:512 * (sub + 1)],
                                    lhsT=wk[:, kc, :],
                                    rhs=xT[:, kc, 512 * sc:512 * (sc + 1)],
                                    start=(kc == 0), stop=(kc == KCT - 1))
                        sl = slice(1024 * sh, 1024 * (sh + 1))
                        nc.vector.tensor_copy(kRep[0:64, 0, sl], ps[0:64, :])
                        nc.vector.tensor_copy(kRep[64:128, 1, sl], ps[64:128, :])
                    nc.sync.dma_start(out=kRep[64:128, 0, :], in_=kRep[0:64, 0, :])
                    nc.sync.dma_start(out=kRep[0:64, 1, :], in_=kRep[64:128, 1, :])

                def q_half(p, h):
                    ps = psS.tile([128, 1024], f32, tag="S", name=f"qp{p}{h}")
                    for sub in range(2):
                        tcn = 2 * h + sub
                        for kc in range(KCT):
                            nc.tensor.matmul(
                                ps[:, 512 * sub:512 * (sub + 1)],
                                lhsT=wq[:, kc, 128 * p:128 * (p + 1)],
                                rhs=xT[:, kc, 512 * tcn:512 * (tcn + 1)],
                                start=(kc == 0), stop=(kc == KCT - 1))
                    sl = slice(1024 * h, 1024 * (h + 1))
                    nc.vector.tensor_copy(qRep[0:64, 2 * p, sl], ps[0:64, :])
                    nc.vector.tensor_copy(qRep[64:128, 2 * p + 1, sl], ps[64:128, :])
                    if h == 1:
                        nc.sync.dma_start(out=qRep[64:128, 2 * p, :],
                                          in_=qRep[0:64, 2 * p, :])
                        nc.sync.dma_start(out=qRep[0:64, 2 * p + 1, :],
                                          in_=qRep[64:128, 2 * p + 1, :])

                def v_half(h):
                    ps = psS.tile([128, 1024], f32, tag="S", name=f"vh{h}")
                    for b in range(8):
                        st = 8 * h + b
                        for kc in range(KCT):
                            nc.tensor.matmul(
                                ps[:, 128 * b:128 * (b + 1)],
                                lhsT=xT[:, kc, 128 * st:128 * (st + 1)],
                                rhs=wv[:, kc, :],
                                start=(kc == 0), stop=(kc == KCT - 1))
                    ps3 = ps.rearrange("p (s d) -> p s d", d=128)
                    sl = slice(8 * h, 8 * (h + 1))
                    nc.vector.tensor_copy(v_sb[:, sl, 0:64], ps3[:, :, 0:64])
                    nc.vector.tensor_copy(v_sb[:, sl, 65:129], ps3[:, :, 64:128])
                    nc.vector.memset(v_sb[:, sl, 64], 1.0)
                    nc.vector.memset(v_sb[:, sl, 129], 1.0)

                RR = {}

                def head_attn(i, psAp):
                    p, half = i // 2, i % 2
                    pa = psAp.tile([65, T], f32, tag="pa", name=f"pa{i}")
                    # diag multiplier min(exp(-a(t_in-127)), exp(a(127-s_in)))
                    dmin = work.tile([128, 128], bf16, tag="dmin",
                                     name=f"dm{i}")
                    nc.vector.tensor_scalar(dmin, wrep[:, i, 0:128],
                                            usb[:, i:i + 1], None, MIN)
                    for j in range(ST):
                        W, W5 = _W128[i][j], _W512[i][j]
                        nchunks = (W + 511) // 512
                        E = ebufp.tile([128, T], bf16, tag="E", name=f"E{i}{j}")
                        for sh in range((nchunks + 1) // 2):
                            c0, c1 = 2 * sh, min(nchunks, 2 * sh + 2)
                            S = psS.tile([128, 1024], f32, tag="S",
                                         name=f"S{i}_{j}_{sh}")
                            for c in range(c0, c1):
                                rh = 64 * (c % 2)
                                o = 512 * (c - c0)
                                n = min(512, W - 512 * c)
                                nc.tensor.matmul(
                                    S[:, o:o + n],
                                    lhsT=kRep[rh:rh + 64, half,
                                              128 * j:128 * (j + 1)],
                                    rhs=qRep[rh:rh + 64, i, 512 * c:512 * c + n],
                                    start=True, stop=True)
                            wv_ = min(1024, W - 1024 * sh)
                            nc.scalar.activation(
                                E[:, 1024 * sh:1024 * sh + wv_], S[:, :wv_],
                                AF.Exp, bias=biassb[:, i:i + 1], scale=0.125)
                        if W5 > W:
                            nc.vector.memset(E[:, W:W5], 0.0)
                        lo = 128 * j         # t < lo : future region (mult u)
                        hi = 128 * (j + 1)   # t >= hi: past region (Toeplitz)
                        if lo > 0:
                            nc.vector.tensor_scalar(E[:, :lo], E[:, :lo],
                                                    usb[:, i:i + 1], None, MUL)
                        nc.vector.tensor_tensor(E[:, lo:hi], E[:, lo:hi], dmin,
                                                MUL)
                        if W > hi:
                            nc.vector.tensor_tensor(
                                E[:, hi:W], E[:, hi:W],
                                wrep[:, i, 128:128 + (W - hi)], MUL)
                        for tcn in range(W5 // 512):
                            nc.tensor.matmul(
                                pa[:, 512 * tcn:512 * (tcn + 1)],
                                lhsT=v_sb[:, j, 65 * half:65 * half + 65],
                                rhs=E[:, 512 * tcn:512 * (tcn + 1)],
                                start=(j == _J_FIRST[i][tcn]), stop=(j == ST - 1),
                                skip_group_check=True)
                        yield
                    # ---- per-head epilogue: copy out + reciprocal of denom ----
                    st65 = stp.tile([65, T], bf16, tag="st65", name=f"st{i}")
                    nc.vector.tensor_copy(st65, pa[0:65, :])
                    nc.sync.dma_start(out=outT[64 * half:64 * half + 64, p, :],
                                      in_=st65[0:64, :])
                    nc.sync.dma_start(out=drow_d[i:i + 1, :], in_=st65[64:65, :])
                    dp = dpk.tile([16, 128], bf16, tag="dp", name=f"dp{i}")
                    nc.gpsimd.dma_start(
                        out=dp, in_=drow_d[i].rearrange("(a b) -> a b", b=128))
                    rpf = dpk.tile([16, 128], f32, tag="rpf", name=f"rpf{i}")
                    nc.vector.reciprocal(rpf, dp)
                    rp = dpk.tile([16, 128], bf16, tag="rp", name=f"rp{i}")
                    nc.vector.tensor_copy(rp, rpf)
                    nc.gpsimd.dma_start(
                        out=rrow_d[i].rearrange("(a b) -> a b", b=128), in_=rp)
                    if half == 0:
                        RR[p] = rrp.tile([128, T], bf16, tag="rr", name=f"rr{p}")
                    nc.gpsimd.dma_start(out=RR[p][64 * half:64 * half + 64, :],
                                        in_=bcast(rrow_d[i:i + 1, :], 64))
                    if half == 1:
                        nc.vector.tensor_tensor(outT[:, p, :], outT[:, p, :],
                                                RR[p], MUL)
                    yield

                def drive(gen, inserts):
                    for step, _ in enumerate(gen):
                        if step in inserts:
                            for fn in inserts[step]:
                                fn()

                # ---- emission: k -> q0 -> v -> heads with interleaved proj ----
                k_proj()
                q_half(0, 0)
                q_half(0, 1)
                v_half(0)
                ilv = {
                    0: {1: [lambda: v_half(1)]},
                    1: {4: [lambda: q_half(1, 0)], 8: [lambda: q_half(1, 1)]},
                    3: {4: [lambda: q_half(2, 0)], 8: [lambda: q_half(2, 1)]},
                    5: {4: [lambda: q_half(3, 0)], 8: [lambda: q_half(3, 1)]},
                }
                for i in range(NH):
                    drive(head_attn(i, psAp), ilv.get(i, {}))

            # ---- output projection (psP reuses psA's freed banks) ----
            with tc.tile_pool(name="psP", bufs=4, space="PSUM") as psP:
                for tt in range(ST):
                    osb = outp.tile([128, C], f32, tag="osb", name=f"ob{tt}")
                    for ec in range(2):
                        ps = psP.tile([128, 512], f32, tag="pp",
                                      name=f"pp{tt}_{ec}")
                        for kt in range(4):
                            nc.tensor.matmul(
                                ps, lhsT=outT[:, kt, 128 * tt:128 * (tt + 1)],
                                rhs=wp[:, kt, 512 * ec:512 * (ec + 1)],
                                start=(kt == 0), stop=(kt == 3))
                        nc.vector.tensor_copy(osb[:, 512 * ec:512 * (ec + 1)], ps)
                    eng = nc.sync if tt % 2 == 0 else nc.scalar
                    eng.dma_start(out=out_d[128 * tt:128 * (tt + 1), :],
                                  in_=osb)

    _split_multiwait(nc, mybir)
    _NC_CACHE["nc"] = nc
    return nc


def _prep_core_inputs(x, Wq, Wkv, Wproj, b, g):
    import ml_dtypes
    bf = ml_dtypes.bfloat16
    heads = [_head_of_slot(i, g) for i in range(NH)]
    xT = np.ascontiguousarray(x[b].T).astype(bf)                      # [C, T]
    wq_cols = np.concatenate([Wq[64 * h:64 * (h + 1)] for h in heads], axis=0)
    wqT = np.ascontiguousarray(wq_cols.T).astype(bf)                  # [C, 512]
    wkT = np.ascontiguousarray(Wkv[128 * g:128 * (g + 1)].T).astype(bf)
    wvT = np.ascontiguousarray(Wkv[256 + 128 * g:256 + 128 * (g + 1)].T).astype(bf)
    cols = np.concatenate([np.arange(64 * h, 64 * (h + 1)) for h in heads])
    wpT = np.ascontiguousarray(Wproj[:, cols].T).astype(bf)           # [512, C]

    s_in = np.arange(128, dtype=np.float64)
    wrow = np.empty((NH, WREP_W), dtype=bf)
    u = np.empty((128, NH), dtype=np.float32)
    bias = np.empty((128, NH), dtype=np.float32)
    idx = np.arange(WREP_W, dtype=np.float64)
    for i, h in enumerate(heads):
        a = _a_of_head(h)
        wrow[i] = np.exp(-a * (idx - 127.0)).astype(np.float32)
        u[:, i] = np.exp(a * (127.0 - s_in)).astype(np.float32)
        bias[:, i] = (a * (s_in - 127.0)).astype(np.float32)
    return {"xT": xT, "wqT": wqT, "wkT": wkT, "wvT": wvT, "wpT": wpT,
            "wrow": wrow, "usb": u, "biassb": bias}


def kernel(x, Wq, Wkv, Wproj, bproj):
    from concourse.bass_utils import run_bass_kernel_spmd
    x = np.asarray(x, dtype=np.float32)
    Wq = np.asarray(Wq, dtype=np.float32)
    Wkv = np.asarray(Wkv, dtype=np.float32)
    Wproj = np.asarray(Wproj, dtype=np.float32)
    bproj = np.asarray(bproj, dtype=np.float32)

    nc = _build_nc()
    in_maps = [_prep_core_inputs(x, Wq, Wkv, Wproj, c // 2, c % 2)
               for c in range(8)]
    res = run_bass_kernel_spmd(nc, in_maps, core_ids=list(range(8)))
    out = np.zeros((B, T, C), dtype=np.float32)
    for c in range(8):
        out[c // 2] += res.results[c]["out"]
    out += bproj[None, None, :]
    return out

